# revision 25
# baseline (speedup 1.0000x reference)
"""Trainium2 Bass kernel for a dense transformer block (nn_Block_88158498717701).

Strategy (8 NeuronCores, SPMD single program):
  - Head-parallel attention: core c owns heads {2c, 2c+1}. QKV projections and
    attention run in "transposed space" ([feature, token] layouts), so no
    on-device transposes are needed:
      QT/KT = W{q,k}^T @ x^T  ([128=2*hd, 4096 tokens])
      V natural = x @ Wv      ([4096 tokens, 128=2*hd], via xT-stationary mm)
      S^T = K Q^T per head; exp over 4-PSUM-bank batches on ScalarE;
      causal masking via GPSIMD affine_select on the diagonal-crossing tiles;
      attn^T_h = [V_h | 1]^T @ P^T  (M=65 matmul; row 64 = softmax denom).
      The softmax division is deferred to after the AllToAll.
  - AllToAll redistributes attn^T (+denominator rows) so core c ends with the
    full-D attn^T for its 512-token chunk; divide + residual + LN1 + FFN +
    residual + LN2 are token-parallel in transposed space (LN stats and
    partition-broadcasts via small matmuls).
  - All matmuls run in float32r (fp32 storage, ~tf32 multiply precision,
    ~1 cycle/row on the PE for N>=256).
"""

import numpy as np

import concourse.bacc as bacc
import concourse.mybir as mybir
import concourse.tile as tile
from concourse import bass_utils

F32 = mybir.dt.float32
F32R = mybir.dt.float32r
BF16 = mybir.dt.bfloat16
AF = mybir.ActivationFunctionType
ALU = mybir.AluOpType

N_CORES = 8
D = 1024
T = 2048
B = 2
TOK = B * T
CHUNK = TOK // N_CORES   # 512
HD = 64
DB = D // 128            # 8
HID = 4 * D
HB = HID // 128          # 32
QB = 512
NQB = T // QB            # 4
KBT = T // 128           # 16 k-blocks per batch
SROW = 128               # a2a shard rows (attn only; pre-divided)
LN_EPS = 1e-5

_CACHE = {}


def _build():
    nc = bacc.Bacc("TRN2", target_bir_lowering=False, debug=False,
                   num_devices=N_CORES)

    xT = nc.dram_tensor("xT", [D, TOK], BF16, kind="ExternalInput")
    xtc = nc.dram_tensor("xtc", [D, CHUNK], F32R, kind="ExternalInput")
    wq = nc.dram_tensor("wq", [D, 128], BF16, kind="ExternalInput")
    wk = nc.dram_tensor("wk", [D, 128], BF16, kind="ExternalInput")
    wv = nc.dram_tensor("wv", [D, 128], BF16, kind="ExternalInput")
    bq = nc.dram_tensor("bq", [128, 1], F32, kind="ExternalInput")
    bk = nc.dram_tensor("bk", [128, 1], F32, kind="ExternalInput")
    bv = nc.dram_tensor("bv", [1, 128], F32R, kind="ExternalInput")
    bvc = nc.dram_tensor("bvc", [128, 1], F32, kind="ExternalInput")
    w1 = nc.dram_tensor("w1", [D, HID], F32R, kind="ExternalInput")
    w2 = nc.dram_tensor("w2", [HID, D], F32R, kind="ExternalInput")
    b1s = nc.dram_tensor("b1s", [128, HB], F32, kind="ExternalInput")
    b2s = nc.dram_tensor("b2s", [128, DB], F32, kind="ExternalInput")
    ln1g = nc.dram_tensor("ln1g", [128, DB], F32, kind="ExternalInput")
    ln1b = nc.dram_tensor("ln1b", [128, DB], F32, kind="ExternalInput")
    ln2g = nc.dram_tensor("ln2g", [128, DB], F32, kind="ExternalInput")
    ln2b = nc.dram_tensor("ln2b", [128, DB], F32, kind="ExternalInput")
    outT = nc.dram_tensor("outT", [D, CHUNK], F32, kind="ExternalOutput")

    ones_1x128 = nc.inline_tensor(np.ones((1, 128), np.float32), name="c_ones")
    onescol = nc.inline_tensor(np.ones((128, TOK // 128), np.float32),
                               name="c_onescol")
    ident_c = nc.inline_tensor(np.eye(128, dtype=np.float32), name="c_ident")
    import ml_dtypes
    nmw_bf_c = nc.inline_tensor(
        np.full((128, 1), -1.0 / D, ml_dtypes.bfloat16), name="c_nmwbf")
    meanw = nc.inline_tensor(
        np.concatenate([np.full((128, 1), -1.0 / D, np.float32),
                        np.full((128, 1), 1.0 / D, np.float32)], axis=1),
        name="c_meanw")

    xT_v = xT.ap().rearrange("(r p) t -> p r t", p=128)
    wq_v = wq.ap().rearrange("(r p) m -> p r m", p=128)
    wk_v = wk.ap().rearrange("(r p) m -> p r m", p=128)
    wv_v = wv.ap().rearrange("(r p) m -> p r m", p=128)
    w1_v = w1.ap().rearrange("(r p) h -> p r h", p=128)
    w2_v = w2.ap().rearrange("(r p) d -> p r d", p=128)
    xtc_v = xtc.ap().rearrange("(r p) t -> p r t", p=128)
    outT_v = outT.ap().rearrange("(r p) t -> p r t", p=128)

    with tile.TileContext(nc) as tc:
        with (
            tc.tile_pool(name="psum", bufs=1, space="PSUM") as psum,
            tc.tile_pool(name="cst", bufs=1) as cst,
            tc.tile_pool(name="glob", bufs=1) as glob,
            tc.tile_pool(name="ffn_w", bufs=3) as fw,
            tc.tile_pool(name="dram", bufs=1, space="DRAM") as dram,
        ):
            a2a_in = dram.tile([SROW * N_CORES, CHUNK], BF16)
            a2a_out = dram.tile([SROW * N_CORES, CHUNK], BF16)
            a2a_out_v = a2a_out[:].rearrange("(r c) t -> r c t", c=SROW)

            # QKV weights first (attention-critical DMAs lead)
            wq_sb = cst.tile([128, DB, 128], BF16)
            nc.gpsimd.dma_start(wq_sb[:], wq_v)
            wk_sb = cst.tile([128, DB, 128], BF16)
            nc.gpsimd.dma_start(wk_sb[:], wk_v)
            wv_sb = cst.tile([128, DB, 128], BF16)
            nc.gpsimd.dma_start(wv_sb[:], wv_v)
            onesbv = cst.tile([1, 256], F32R)
            nc.sync.dma_start(onesbv[:, 0:128], ones_1x128.ap().bitcast(F32R))
            nc.sync.dma_start(onesbv[:, 128:256], bv.ap())
            ones_sb = onesbv[:, 0:128]
            bv_sb = onesbv[:, 128:256]
            # smalls f32: bq 0, bk 1, b2s 2:10, g1 10:18, be1 18:26,
            # g2 26:34, be2 34:42, eps [0,42]
            smalls = cst.tile([128, 76], F32)
            nc.sync.dma_start(smalls[:, 0:1], bq.ap())
            nc.sync.dma_start(smalls[:, 1:2], bk.ap())
            nc.sync.dma_start(smalls[:, 43:44], bvc.ap())
            bq_sb = smalls[:, 0:1]
            bk_sb = smalls[:, 1:2]
            bvc_sb = smalls[:, 43:44]
            ident = cst.tile([128, 128], BF16)
            nc.gpsimd.dma_start(ident[:], ident_c.ap())

            # h accumulator lives across both phases: preload x^T chunk now
            h_sb = glob.tile([128, DB, CHUNK], F32R)
            nc.gpsimd.dma_start(h_sb[:], xtc_v)

            # ======== Phase 1: QKV projections + attention ========
            with (
                tc.tile_pool(name="attn_big", bufs=1) as abig,
                tc.tile_pool(name="pt_pool", bufs=4) as ptp,
                tc.tile_pool(name="attn_tmp", bufs=2) as atmp,
            ):
                qt_sb = abig.tile([128, TOK], BF16)
                kt_sb = abig.tile([128, TOK], BF16)
                v_sb = abig.tile([128, TOK // 128, 130], BF16)
                nc.gpsimd.dma_start(v_sb[:, :, 64:65],
                                    onescol.ap()[:, :, None])
                nc.gpsimd.dma_start(v_sb[:, :, 129:130],
                                    onescol.ap()[:, :, None])

                def qkv_closures(tb):
                    ts_ = slice(tb * 512, (tb + 1) * 512)
                    state = {}

                    def load():
                        xt_t = atmp.tile([128, DB, 512], BF16, tag="xt")
                        nc.sync.dma_start(xt_t[:], xT_v[:, :, ts_])
                        state["xt"] = xt_t

                    def do_q():
                        xt_t = state["xt"]
                        pq = psum.tile([128, 512], F32, tag="qk", bufs=2)
                        for r in range(DB):
                            nc.tensor.matmul(pq[:], wq_sb[:, r, :],
                                             xt_t[:, r, :],
                                             start=(r == 0),
                                             stop=(r == DB - 1))
                        nc.vector.tensor_scalar_add(qt_sb[:, ts_], pq[:],
                                                    bq_sb)

                    def do_k():
                        xt_t = state["xt"]
                        pk = psum.tile([128, 512], F32, tag="qk", bufs=2)
                        for r in range(DB):
                            nc.tensor.matmul(pk[:], wk_sb[:, r, :],
                                             xt_t[:, r, :],
                                             start=(r == 0),
                                             stop=(r == DB - 1))
                        nc.vector.tensor_scalar_add(kt_sb[:, ts_], pk[:],
                                                    bk_sb)

                    def do_v():
                        xt_t = state["xt"]
                        pvt = psum.tile([128, 512], F32, tag="qk", bufs=2)
                        for r in range(DB):
                            nc.tensor.matmul(pvt[:], wv_sb[:, r, :],
                                             xt_t[:, r, :],
                                             start=(r == 0),
                                             stop=(r == DB - 1))
                        vt_t = atmp.tile([128, 512], BF16, tag="vt")
                        nc.vector.tensor_scalar_add(vt_t[:], pvt[:], bvc_sb)
                        state["vt"] = vt_t

                    def do_vt():
                        vt_t = state["vt"]
                        for sub in range(4):
                            ptr = psum.tile([128, 128], BF16, tag="qk",
                                            bufs=2, name="ptr")
                            nc.tensor.transpose(
                                ptr[:], vt_t[:, sub * 128:(sub + 1) * 128],
                                ident[:])
                            kb = tb * 4 + sub
                            nc.vector.tensor_copy(v_sb[:, kb, 0:64],
                                                  ptr[:, 0:64])
                            nc.vector.tensor_copy(v_sb[:, kb, 65:129],
                                                  ptr[:, 64:128])

                    return [load, do_q, do_k, do_v, do_vt]

                work = []  # pending QKV closures

                def drain_work(n):
                    for _ in range(min(n, len(work))):
                        work.pop(0)()

                def emit_attn_unit(b, qb):
                    qs = slice(b * T + qb * QB, b * T + (qb + 1) * QB)
                    chunk_idx = b * NQB + qb
                    nkb = 4 * (qb + 1)
                    pav = [psum.tile([128, 512], F32, tag="pav", bufs=2,
                                     name=f"pav{h}") for h in range(2)]
                    pend = []

                    def emit_av(item):
                        pt2_, kb_ = item
                        kbg = b * KBT + kb_
                        for h in range(2):
                            vsl = slice(65 * h, 65 * h + 65)
                            nc.tensor.matmul(
                                pav[h][0:65, :], v_sb[:, kbg, vsl],
                                pt2_[:, h * 512:(h + 1) * 512],
                                start=(kb_ == 0), stop=(kb_ == nkb - 1))

                    for kb in range(nkb):
                        kbg = b * KBT + kb
                        ks = slice(kbg * 128, (kbg + 1) * 128)
                        ps2 = psum.tile([128, 1024], F32, tag="s2",
                                        bufs=2, name="ps2")
                        for h in range(2):
                            hs = slice(h * 64, (h + 1) * 64)
                            nc.tensor.matmul(
                                ps2[:, h * 512:(h + 1) * 512],
                                kt_sb[hs, ks], qt_sb[hs, qs],
                                start=True, stop=True)
                        pt2 = ptp.tile([128, 1024], BF16, tag="pt")
                        nc.scalar.activation(pt2[:], ps2[:], AF.Exp,
                                             scale=0.125)
                        kb_rel = kb - 4 * qb
                        if kb_rel >= 0:
                            for h in range(2):
                                nc.gpsimd.affine_select(
                                    pt2[:, h * 512:(h + 1) * 512],
                                    pt2[:, h * 512:(h + 1) * 512],
                                    pattern=[[1, 512]],
                                    compare_op=ALU.is_ge, fill=0.0,
                                    base=-128 * kb_rel,
                                    channel_multiplier=-1)
                        pend.append((pt2, kb))
                        if len(pend) > 2:
                            emit_av(pend.pop(0))
                        pass
                    while pend:
                        emit_av(pend.pop(0))

                    # divide by denominator (row 64) and ship
                    for h in range(2):
                        r0 = chunk_idx * SROW
                        den = atmp.tile([1, 512], F32, tag="den")
                        nc.vector.tensor_copy(den[:], pav[h][64:65, :])
                        rs = atmp.tile([1, 512], F32, tag="rs")
                        nc.vector.reciprocal_approx_fast(rs[:], den[:])
                        rsr = atmp.tile([1, 512], F32R, tag="rsr")
                        nc.vector.tensor_copy(rsr[:], rs[:])
                        pbc = psum.tile([64, 512], F32, tag="s2", bufs=2,
                                        name="pbc")
                        nc.tensor.matmul(pbc[:], ones_sb[:, 0:64], rsr[:],
                                         start=True, stop=True)
                        rb = atmp.tile([64, 512], F32, tag="rb")
                        nc.vector.tensor_copy(rb[:], pbc[:])
                        af = atmp.tile([64, 512], BF16, tag="af")
                        nc.vector.tensor_tensor(af[:], pav[h][0:64, :],
                                                rb[:], ALU.mult)
                        nc.sync.dma_start(
                            a2a_in[r0 + h * 64:r0 + h * 64 + 64, :], af[:])

                units = [(b, qb) for b in range(B) for qb in range(NQB)]
                work.extend(qkv_closures(0))
                drain_work(5)
                work.extend(qkv_closures(1))
                drain_work(5)
                for step in range(2, TOK // 512):
                    work.extend(qkv_closures(step))
                    drain_work(len(work))
                    emit_attn_unit(*units[step - 2])
                for u in range(6, 8):
                    emit_attn_unit(*units[u])

            # ======== AllToAll ========
            nc.gpsimd.collective_compute(
                "AllToAll", ALU.bypass,
                replica_groups=[list(range(N_CORES))],
                ins=[a2a_in[:].opt()],
                outs=[a2a_out[:].opt()],
            )

            # ======== Phase 2: divide + residual + LN1 + FFN + LN2 ========
            with (
                tc.tile_pool(name="ffn_big", bufs=1) as fbig,
                tc.tile_pool(name="ffn_tmp", bufs=2) as ftmp,
                tc.tile_pool(name="stats", bufs=1) as stp,
            ):
                nc.sync.dma_start(smalls[:, 2:2 + DB], b2s.ap())
                nc.sync.dma_start(smalls[:, 44:44 + HB], b1s.ap())
                nc.sync.dma_start(smalls[:, 10:10 + DB], ln1g.ap())
                nc.sync.dma_start(smalls[:, 18:18 + DB], ln1b.ap())
                nc.sync.dma_start(smalls[:, 26:26 + DB], ln2g.ap())
                nc.sync.dma_start(smalls[:, 34:34 + DB], ln2b.ap())
                nc.vector.memset(smalls[0:1, 42:43], LN_EPS)
                b2s_sb = smalls[:, 2:2 + DB]
                b1s_sb = smalls[:, 44:44 + HB]
                g1 = smalls[:, 10:10 + DB]
                be1 = smalls[:, 18:18 + DB]
                g2 = smalls[:, 26:26 + DB]
                be2 = smalls[:, 34:34 + DB]
                eps_sb = smalls[0:1, 42:43]

                nmwb = cst.tile([128, 1], BF16)
                nc.sync.dma_start(nmwb[:], nmw_bf_c.ap())
                meanw_sb = cst.tile([128, 2], F32R)
                nc.sync.dma_start(meanw_sb[:], meanw.ap().bitcast(F32R))
                nmw = meanw_sb[:, 0:1]
                pmw = meanw_sb[:, 1:2]

                zr_sb = fbig.tile([128, HB, CHUNK], F32R)
                # residual 1: h = xtc + attn; LN1 stats interleaved
                st = stp.tile([1, 3 * CHUNK], F32, tag="st")
                nmu_t = stp.tile([1, CHUNK], F32R, tag="nmu")
                rstd_t = stp.tile([1, CHUNK], F32R, tag="rstd")

                def ln_stats(ht, r, pmu, pms):
                    nc.tensor.matmul(pmu[:], nmw, ht[:, r, :],
                                     start=(r == 0), stop=(r == DB - 1))
                    sq = ftmp.tile([128, CHUNK], F32R, tag="sq")
                    nc.scalar.activation(sq[:], ht[:, r, :], AF.Square)
                    nc.tensor.matmul(pms[:], pmw, sq[:],
                                     start=(r == 0), stop=(r == DB - 1))

                def ln_apply(ht, g_sb, b_sb, pmu, pms, out_t=None):
                    nmu = nmu_t[:]
                    musq = st[:, 0:CHUNK]
                    var = st[:, CHUNK:2 * CHUNK]
                    sd = st[:, 2 * CHUNK:3 * CHUNK]
                    rstd = rstd_t[:]
                    nc.vector.tensor_copy(nmu, pmu[:])
                    nc.vector.tensor_tensor(musq, nmu, nmu, ALU.mult)
                    nc.vector.tensor_tensor(var, pms[:], musq, ALU.subtract)
                    nc.scalar.activation(sd, var, AF.Sqrt, bias=eps_sb)
                    with nc.allow_low_precision(reason="ln rstd to f32r"):
                        nc.vector.reciprocal(rstd, sd)
                    pb1 = psum.tile([128, 512], F32, tag="qk", bufs=2,
                                    name="pb1")
                    nc.tensor.matmul(pb1[:], ones_sb, rstd,
                                     start=True, stop=True)
                    pb2 = psum.tile([128, 512], F32, tag="pav", bufs=2,
                                    name="pb2")
                    nc.tensor.matmul(pb2[:], ones_sb, nmu,
                                     start=True, stop=True)
                    dst = ht if out_t is None else out_t
                    bc1 = ftmp.tile([128, CHUNK], F32, tag="bc1")
                    nc.vector.tensor_copy(bc1[:], pb1[:])
                    bc2 = ftmp.tile([128, CHUNK], F32, tag="bc2")
                    nc.vector.tensor_copy(bc2[:], pb2[:])
                    for r in range(DB):
                        eng = nc.vector if r % 2 == 0 else nc.gpsimd
                        t1 = ftmp.tile([128, CHUNK], F32,
                                       tag=f"t1{r % 2}")
                        eng.tensor_tensor(t1[:], ht[:, r, :], bc2[:],
                                          ALU.add)
                        eng.tensor_tensor(t1[:], t1[:], bc1[:], ALU.mult)
                        eng.tensor_scalar(dst[:, r, :], t1[:],
                                          g_sb[:, r:r + 1],
                                          b_sb[:, r:r + 1],
                                          ALU.mult, ALU.add)

                pmu1 = psum.tile([1, 512], F32, tag="s2", bufs=2, name="pmu1")
                pms1 = psum.tile([1, 512], F32, tag="s2", bufs=2, name="pms1")
                for r in range(DB):
                    at_t = ftmp.tile([128, CHUNK], BF16, tag="a2al")
                    nc.sync.dma_start(at_t[:], a2a_out_v[r, 0:128, :])
                    eng = nc.vector if r % 2 == 0 else nc.gpsimd
                    eng.tensor_tensor(h_sb[:, r, :], h_sb[:, r, :],
                                      at_t[:], ALU.add)
                    ln_stats(h_sb, r, pmu1, pms1)
                ln_apply(h_sb, g1, be1, pmu1, pms1)
                for ho in range(HB):
                    w1_t = fw.tile([128, DB, 128], F32R, tag="w1")
                    nc.sync.dma_start(
                        w1_t[:], w1_v[:, :, ho * 128:(ho + 1) * 128])
                    pz = psum.tile([128, 512], F32,
                                   tag=("qk" if ho % 2 else "pav"), bufs=2,
                                   name="pz")
                    for r in range(DB):
                        nc.tensor.matmul(pz[:], w1_t[:, r, :], h_sb[:, r, :],
                                         start=(r == 0), stop=(r == DB - 1))
                    nc.vector.tensor_scalar(zr_sb[:, ho, :], pz[:],
                                            b1s_sb[:, ho:ho + 1],
                                            0.0, ALU.add, ALU.max)

                # FFN2 + residual, written back into h; LN2 stats inline
                pmu2 = psum.tile([1, 512], F32, tag="s2", bufs=2, name="pmu2")
                pms2 = psum.tile([1, 512], F32, tag="s2", bufs=2, name="pms2")
                for do in range(DB):
                    pf = psum.tile([128, 512], F32,
                                   tag=("qk" if do % 2 else "pav"), bufs=2,
                                   name="pf")
                    for hh in range(2):
                        w2_t = ftmp.tile([128, HB // 2, 128], F32R, tag="w2")
                        nc.sync.dma_start(
                            w2_t[:],
                            w2_v[:, hh * 16:(hh + 1) * 16,
                                 do * 128:(do + 1) * 128])
                        for ho in range(HB // 2):
                            hog = hh * 16 + ho
                            nc.tensor.matmul(pf[:], w2_t[:, ho, :],
                                             zr_sb[:, hog, :],
                                             start=(hog == 0),
                                             stop=(hog == HB - 1))
                    tf_ = ftmp.tile([128, CHUNK], F32, tag="tf")
                    nc.vector.tensor_scalar_add(tf_[:], pf[:],
                                                b2s_sb[:, do:do + 1])
                    nc.vector.tensor_tensor(h_sb[:, do, :], tf_[:],
                                            h_sb[:, do, :], ALU.add)
                    ln_stats(h_sb, do, pmu2, pms2)
                out_sb = fbig.tile([128, DB, CHUNK], F32)
                ln_apply(h_sb, g2, be2, pmu2, pms2, out_t=out_sb)
                for r in range(DB):
                    nc.sync.dma_start(outT_v[:, r, :], out_sb[:, r, :])

    nc.compile()
    return nc


def _get_nc():
    if "nc" not in _CACHE:
        _CACHE["nc"] = _build()
    return _CACHE["nc"]


def _in_maps(x, W_qkv, b_qkv, W1, b1, W2, b2, ln1_g, ln1_b, ln2_g, ln2_b):
    import ml_dtypes
    bf16 = ml_dtypes.bfloat16
    f32 = np.float32
    x = np.asarray(x, f32)
    flat = x.reshape(TOK, D)
    xT = np.ascontiguousarray(flat.T)
    xT_bf = xT.astype(bf16)
    Wr = np.asarray(W_qkv, f32).reshape(D, 3, 16, HD)
    br = np.asarray(b_qkv, f32).reshape(3, 16, HD)
    W1f = np.ascontiguousarray(np.asarray(W1, f32))
    W2f = np.ascontiguousarray(np.asarray(W2, f32))
    b1sv = np.ascontiguousarray(np.asarray(b1, f32).reshape(HB, 128).T)
    b2sv = np.ascontiguousarray(np.asarray(b2, f32).reshape(DB, 128).T)
    g1v = np.ascontiguousarray(np.asarray(ln1_g, f32).reshape(DB, 128).T)
    be1v = np.ascontiguousarray(np.asarray(ln1_b, f32).reshape(DB, 128).T)
    g2v = np.ascontiguousarray(np.asarray(ln2_g, f32).reshape(DB, 128).T)
    be2v = np.ascontiguousarray(np.asarray(ln2_b, f32).reshape(DB, 128).T)
    maps = []
    for c in range(N_CORES):
        hsl = slice(2 * c, 2 * c + 2)
        maps.append({
            "xT": xT_bf,
            "xtc": np.ascontiguousarray(xT[:, c * CHUNK:(c + 1) * CHUNK]),
            "wq": np.ascontiguousarray(Wr[:, 0, hsl, :].reshape(D, 128)).astype(bf16),
            "wk": np.ascontiguousarray(Wr[:, 1, hsl, :].reshape(D, 128)).astype(bf16),
            "wv": np.ascontiguousarray(Wr[:, 2, hsl, :].reshape(D, 128)).astype(bf16),
            "bq": np.ascontiguousarray(br[0, hsl].reshape(128, 1)),
            "bk": np.ascontiguousarray(br[1, hsl].reshape(128, 1)),
            "bv": np.ascontiguousarray(br[2, hsl].reshape(1, 128)),
            "bvc": np.ascontiguousarray(br[2, hsl].reshape(128, 1)),
            "w1": W1f, "w2": W2f, "b1s": b1sv, "b2s": b2sv,
            "ln1g": g1v, "ln1b": be1v, "ln2g": g2v, "ln2b": be2v,
        })
    return maps


def kernel(**inputs) -> np.ndarray:
    nc = _get_nc()
    maps = _in_maps(**inputs)
    res = bass_utils.run_bass_kernel_spmd(nc, maps,
                                          core_ids=list(range(N_CORES)))
    out_flat = np.empty((TOK, D), np.float32)
    for c in range(N_CORES):
        out_flat[c * CHUNK:(c + 1) * CHUNK, :] = res.results[c]["outT"].T
    return out_flat.reshape(B, T, D)
